# revision 1
# baseline (speedup 1.0000x reference)
"""DeltaNet forward kernel for Trainium2, sharded over 8 NeuronCores.

Sharding: core c handles batch c//2 and head-pair c%2 (heads {2*(c%2), 2*(c%2)+1}).
Each core computes: causal depthwise conv+silu, q/k/v/beta/g projections for its
head pair, the delta-rule recurrence via the chunked WY representation
(chunk=128, (I+A)^-1 via Neumann-series doubling), gated RMSNorm, and a partial
output projection against its 512-column slice of Wo. The host sums the two
half-DV partials per batch (row-parallel unshard).
"""

import sys

for _p in ("/opt/trn_rl_repo", "/root/.axon_site"):
    if _p not in sys.path:
        sys.path.insert(0, _p)

import numpy as np

import concourse.bass as bass
import concourse.tile as tile
from concourse import bacc, mybir
from concourse.bass_utils import run_bass_kernel_spmd
from concourse.masks import make_identity

F32 = mybir.dt.float32
F32R = mybir.dt.float32r
BF16 = mybir.dt.bfloat16

B, L, D, H = 4, 2048, 1024, 4
DK, DV = 512, 1024
HK, HV = 128, 256
CONV, EPS = 4, 1e-5
C = 128            # delta-rule chunk length
NCH = L // C       # 16 chunks
LB = 256           # L-block for projections
NLB = L // LB      # 4
KD = D // 128      # 8 contraction slices
HPC = 2            # heads per core
N_CORES = 8
QSCALE = HK ** -0.5
NEUMANN16 = True   # Tinv = sum_{k<16} M^k (else k<8)


def _mm(nc, out, lhsT, rhs, start, stop):
    """float32r matmul (full-rate 1 cycle/row when moving free dim >= 256).
    Operand tiles must be declared float32r so their producers round."""
    assert lhsT.dtype == F32R and rhs.dtype == F32R, (lhsT.dtype, rhs.dtype)
    nc.tensor.matmul(out, lhsT, rhs, start=start, stop=stop)


def build_program():
    nc = bacc.Bacc(
        "TRN2", target_bir_lowering=False, debug=False,
        enable_asserts=False, num_devices=N_CORES,
    )

    hs = nc.dram_tensor("hs", [L, D], F32, kind="ExternalInput").ap()
    cw = nc.dram_tensor("cw", [D, CONV], F32, kind="ExternalInput").ap()
    wq = nc.dram_tensor("wq", [HPC * HK, D], F32, kind="ExternalInput").ap()
    wk = nc.dram_tensor("wk", [HPC * HK, D], F32, kind="ExternalInput").ap()
    wv = nc.dram_tensor("wv", [HPC * HV, D], F32, kind="ExternalInput").ap()
    wb = nc.dram_tensor("wb", [HPC, D], F32, kind="ExternalInput").ap()
    wg = nc.dram_tensor("wg", [HPC * HV, D], F32, kind="ExternalInput").ap()
    wo = nc.dram_tensor("wo", [D, HPC * HV], F32, kind="ExternalInput").ap()
    rmsw = nc.dram_tensor("rmsw", [HV], F32, kind="ExternalInput").ap()
    y = nc.dram_tensor("y", [L, D], F32, kind="ExternalOutput").ap()

    with tile.TileContext(nc) as tc:
        _build_body(nc, tc, hs, cw, wq, wk, wv, wb, wg, wo, rmsw, y)
    nc.compile()
    return nc


def _build_body(nc, tc, hs, cw, wq, wk, wv, wb, wg, wo, rmsw, y):
    from contextlib import ExitStack

    ctx = ExitStack()
    const = ctx.enter_context(tc.tile_pool(name="const", bufs=1))
    wT = ctx.enter_context(tc.tile_pool(name="wT", bufs=1))
    wrow = ctx.enter_context(tc.tile_pool(name="wrow", bufs=3))
    dpool = ctx.enter_context(tc.tile_pool(name="dpool", bufs=2))
    ps = ctx.enter_context(tc.tile_pool(name="ps", bufs=8, space="PSUM"))
    hpool = ctx.enter_context(tc.tile_pool(name="hpool", bufs=2))
    scr = ctx.enter_context(tc.tile_pool(name="scr", bufs=3))
    xpool = ctx.enter_context(tc.tile_pool(name="xpool", bufs=2))
    hrow = ctx.enter_context(tc.tile_pool(name="hrow", bufs=2))
    qk = ctx.enter_context(tc.tile_pool(name="qk", bufs=2))
    ck = ctx.enter_context(tc.tile_pool(name="ck", bufs=3))
    ckx = ctx.enter_context(tc.tile_pool(name="ckx", bufs=6))
    otp = ctx.enter_context(tc.tile_pool(name="otp", bufs=3))
    cv = ctx.enter_context(tc.tile_pool(name="cv", bufs=3))
    sS = ctx.enter_context(tc.tile_pool(name="sS", bufs=4))
    sm = ctx.enter_context(tc.tile_pool(name="sm", bufs=6))

    # copy PSUM->SBUF on alternating engines to balance ACT/DVE load
    cp_state = [0]

    def copy_ps(dst, src):
        cp_state[0] ^= 1
        if cp_state[0]:
            nc.scalar.copy(dst, src)
        else:
            nc.vector.tensor_copy(dst, src)

    ident = const.tile([128, 128], F32)
    make_identity(nc, ident)
    epst = const.tile([128, 1], F32)
    nc.vector.memset(epst, EPS)
    identb = const.tile([128, 128], BF16)
    make_identity(nc, identb)
    # umask: 1 where free >= part (upper incl diag); numask: -1 where free > part
    umask = const.tile([128, 128], F32)
    nc.gpsimd.memset(umask, 1.0)
    nc.gpsimd.affine_select(
        out=umask, in_=umask, compare_op=mybir.AluOpType.is_ge, fill=0.0,
        base=0, channel_multiplier=-1, pattern=[[1, 128]],
    )
    numask = const.tile([128, 128], F32)
    nc.gpsimd.memset(numask, -1.0)
    nc.gpsimd.affine_select(
        out=numask, in_=numask, compare_op=mybir.AluOpType.is_gt, fill=0.0,
        base=0, channel_multiplier=-1, pattern=[[1, 128]],
    )

    def transpose_f32(in_):
        pt = ps.tile([128, 128], F32, tag="ps")
        nc.tensor.transpose(pt, in_, ident[: in_.shape[0], : in_.shape[0]])
        return pt

    # ---- constant loads ----
    cwt = const.tile([128, KD * CONV], F32)
    for d in range(KD):
        nc.sync.dma_start(
            out=cwt[:, d * CONV:(d + 1) * CONV], in_=cw[d * 128:(d + 1) * 128, :]
        )
    rmsc = const.tile([128, 2], F32)
    for s in range(2):
        nc.sync.dma_start(
            out=rmsc[:, s:s + 1],
            in_=rmsw[s * 128:(s + 1) * 128].rearrange("(p one) -> p one", one=1),
        )

    # ---- transposed weights ----
    wqT = wT.tile([128, KD, HPC * HK], F32R)   # q weights^T, pre-scaled by HK^-0.5
    wkT = wT.tile([128, KD, HPC * HK], F32R)
    wvbT = wT.tile([128, KD, HV + HPC], F32R)  # [0:256]=v head0, [256:258]=beta both
    wvT1 = wT.tile([128, KD, HV], F32R)        # v head1
    wgT = wT.tile([128, KD, HPC * HV], F32R)
    woT = wT.tile([128, 4, D], F32R)           # rms_weight folded in

    for rt in range(HPC * HK // 128):  # wq, wk: 2 row tiles each
        wr = wrow.tile([128, D], F32, tag="wrow")
        nc.sync.dma_start(out=wr, in_=wq[rt * 128:(rt + 1) * 128, :])
        wr2 = wrow.tile([128, D], F32, tag="wrow")
        nc.sync.dma_start(out=wr2, in_=wk[rt * 128:(rt + 1) * 128, :])
        for d in range(KD):
            pt = transpose_f32(wr[:, d * 128:(d + 1) * 128])
            nc.scalar.mul(wqT[:, d, rt * 128:(rt + 1) * 128], pt, QSCALE)
            pt2 = transpose_f32(wr2[:, d * 128:(d + 1) * 128])
            copy_ps(wkT[:, d, rt * 128:(rt + 1) * 128], pt2)

    for rt in range(HPC * HV // 128):  # wv: 4 row tiles
        wr = wrow.tile([128, D], F32, tag="wrow")
        nc.sync.dma_start(out=wr, in_=wv[rt * 128:(rt + 1) * 128, :])
        for d in range(KD):
            pt = transpose_f32(wr[:, d * 128:(d + 1) * 128])
            if rt < 2:
                copy_ps(wvbT[:, d, rt * 128:(rt + 1) * 128], pt)
            else:
                copy_ps(wvT1[:, d, (rt - 2) * 128:(rt - 1) * 128], pt)

    wrb = const.tile([HPC, D], F32)
    nc.sync.dma_start(out=wrb, in_=wb)
    for d in range(KD):
        pt = ps.tile([128, HPC], F32, tag="ps")
        nc.tensor.transpose(pt, wrb[:, d * 128:(d + 1) * 128], ident[:HPC, :HPC])
        copy_ps(wvbT[:, d, HV:HV + HPC], pt)

    for rt in range(HPC * HV // 128):  # wg: 4 row tiles
        wr = wrow.tile([128, D], F32, tag="wrow")
        nc.sync.dma_start(out=wr, in_=wg[rt * 128:(rt + 1) * 128, :])
        for d in range(KD):
            pt = transpose_f32(wr[:, d * 128:(d + 1) * 128])
            copy_ps(wgT[:, d, rt * 128:(rt + 1) * 128], pt)

    for rt in range(KD):  # wo: 8 row tiles of [128, 512]
        wr = wrow.tile([128, HPC * HV], F32, tag="wrow")
        nc.sync.dma_start(out=wr, in_=wo[rt * 128:(rt + 1) * 128, :])
        for s in range(4):
            pt = transpose_f32(wr[:, s * 128:(s + 1) * 128])
            nc.vector.tensor_scalar_mul(
                woT[:, s, rt * 128:(rt + 1) * 128], pt, rmsc[:, (s % 2):(s % 2) + 1]
            )

    # ---- state ----
    z256 = const.tile([128, HV], F32)
    nc.vector.memset(z256, 0.0)
    S = []
    for h in range(HPC):
        st = sS.tile([128, HV], F32R, tag="S")
        nc.scalar.copy(st, z256)
        S.append(st)

    AF = mybir.AluOpType
    ACT = mybir.ActivationFunctionType

    lbstate = {"prev_hT": None}

    def stage_lb(lb):
        # hT block: [:, d, 8:8+LB] = fresh transposed h; [:, d, 5:8] = prev tail
        hT = hpool.tile([128, KD, LB + 8], F32R, tag="hT")
        if lb > 0:
            nc.vector.tensor_copy(hT[:, :, 5:8], lbstate["prev_hT"][:, :, LB + 5:LB + 8])
        for lt in range(LB // 128):
            hr = hrow.tile([128, D], F32, tag="hrow")
            row = lb * (LB // 128) + lt
            nc.sync.dma_start(out=hr, in_=hs[row * 128:(row + 1) * 128, :])
            for d in range(KD):
                pt = transpose_f32(hr[:, d * 128:(d + 1) * 128])
                copy_ps(hT[:, d, 8 + lt * 128:8 + (lt + 1) * 128], pt)
        lbstate["prev_hT"] = hT

        # conv + silu -> xT block [128, KD, LB]
        xT = xpool.tile([128, KD, LB], F32R, tag="xT")
        for d in range(KD):
            dg = dpool.tile([128, CONV, 128], F32R, tag="dg")
            for j in range(CONV):
                nc.scalar.mul(dg[:, j, :], ident, cwt[:, d * CONV + j:d * CONV + j + 1])
            pc = ps.tile([128, LB], F32, tag="ps")
            if lb == 0:
                # first block: clip the shifted taps instead of zero-padding
                _mm(nc, pc, dg[:, 3, :], hT[:, d, 8:8 + LB], start=True, stop=False)
                for j in range(3):
                    nc.tensor.matmul(
                        pc[:, 3 - j:LB], dg[:, j, :].bitcast(F32),
                        hT[:, d, 8:8 + LB - (3 - j)].bitcast(F32),
                        start=False, stop=(j == 2))
            else:
                for j in range(CONV):
                    _mm(nc, pc, dg[:, j, :], hT[:, d, 5 + j:5 + j + LB],
                        start=(j == 0), stop=(j == CONV - 1))
            nc.scalar.activation(xT[:, d, :], pc, ACT.Silu)

        # q/k projections (T layout) for this L-block
        qT = qk.tile([128, HPC, LB], F32R, tag="qT")
        kT = qk.tile([128, HPC, LB], F32, tag="kT")
        for h in range(HPC):
            for (wt, dst) in ((wqT, qT), (wkT, kT)):
                pp = ps.tile([128, LB], F32, tag="ps")
                for ks in range(KD):
                    _mm(nc, pp, wt[:, ks, h * 128:(h + 1) * 128], xT[:, ks, :],
                        start=(ks == 0), stop=(ks == KD - 1))
                copy_ps(dst[:, h, :], pp)
        return qT, kT, xT

    def stage_a(c, qT, kT, xT):
        """Chunk-parallel work: v/g/beta projections, k-norm, A/Mqk, TinvT, -W^T."""
        ch = c % (LB // C)
        csl = slice(ch * C, (ch + 1) * C)

        pv0 = ps.tile([128, HV + HPC], F32, tag="ps")
        pv1 = ps.tile([128, HV], F32, tag="ps")
        pg = ps.tile([128, HPC * HV], F32, tag="ps")
        for ks in range(KD):
            lx = xT[:, ks, csl]
            _mm(nc, pv0, lx, wvbT[:, ks, :], start=(ks == 0), stop=(ks == KD - 1))
            _mm(nc, pv1, lx, wvT1[:, ks, :], start=(ks == 0), stop=(ks == KD - 1))
            _mm(nc, pg, lx, wgT[:, ks, :], start=(ks == 0), stop=(ks == KD - 1))
        beta = sm.tile([128, HPC], F32, tag="beta")
        nc.scalar.activation(beta, pv0[:, HV:HV + HPC], ACT.Sigmoid)
        sg = cv.tile([128, HPC * HV], F32, tag="sg")
        nc.scalar.activation(sg, pg, ACT.Silu)
        vb = cv.tile([128, HPC * HV], F32R, tag="vb")
        nc.vector.tensor_scalar_mul(vb[:, 0:HV], pv0[:, 0:HV], beta[:, 0:1])
        nc.vector.tensor_scalar_mul(vb[:, HV:2 * HV], pv1, beta[:, 1:2])

        art = {"vb": vb, "sg": sg, "qT": qT, "csl": csl, "h": []}
        for h in range(HPC):
            # --- k normalization (row space) ---
            pt = transpose_f32(kT[:, h, csl])
            kraw = ck.tile([128, 128], F32, tag="kraw")
            copy_ps(kraw, pt)
            sq = scr.tile([128, 128], F32, tag="sq")
            nsq = sm.tile([128, 1], F32, tag="nsq")
            nc.scalar.activation(sq, kraw, ACT.Square, accum_out=nsq)
            nrm = sm.tile([128, 1], F32, tag="nrm")
            nc.scalar.sqrt(nrm, nsq)
            nrm2 = sm.tile([128, 1], F32, tag="nrm2")
            nc.vector.tensor_scalar_max(nrm2, nrm, 1e-6)
            inv = sm.tile([128, 1], F32, tag="inv")
            nc.vector.reciprocal(inv, nrm2)
            knr = ckx.tile([128, 128], F32R, tag="knr")   # Kn row [C, HK]
            nc.vector.tensor_scalar_mul(knr, kraw, inv)
            kbr = ck.tile([128, 128], F32, tag="kbr")    # beta*Kn row
            nc.vector.tensor_scalar_mul(kbr, knr.bitcast(F32), beta[:, h:h + 1])
            pt = transpose_f32(knr.bitcast(F32))
            knT = ck.tile([128, 128], F32, tag="knT")
            copy_ps(knT, pt)
            pt = transpose_f32(kbr)
            kbT = ck.tile([128, 128], F32, tag="kbT")
            copy_ps(kbT, pt)

            # --- A^T = Kn Kb^T ; Mqk^T = masked Kn Q^T ---
            pA = ps.tile([128, 128], F32, tag="ps")
            nc.tensor.matmul(pA, knT, kbT, start=True, stop=True)
            pM = ps.tile([128, 128], F32, tag="ps")
            nc.tensor.matmul(pM, knT, qT[:, h, csl].bitcast(F32), start=True, stop=True)
            mqk = ckx.tile([128, 128], F32R, tag="mqk")
            nc.vector.tensor_mul(mqk, pM, umask)

            # --- TinvT = sum_k M^k, M = strict_upper(-A^T), bf16 doubling ---
            Mb = ck.tile([128, 128], BF16, tag="Mb")
            nc.vector.tensor_mul(Mb, pA, numask)
            S2 = ck.tile([128, 128], BF16, tag="S2")
            nc.vector.tensor_add(S2, Mb, identb)
            pt = ps.tile([128, 128], BF16, tag="ps")
            nc.tensor.transpose(pt, Mb, identb)
            Nb = ck.tile([128, 128], BF16, tag="Nb")
            copy_ps(Nb, pt)

            def mmb(lhsT, rhs):
                po = ps.tile([128, 128], F32, tag="ps")
                nc.tensor.matmul(po, lhsT, rhs, start=True, stop=True)
                return po

            def cast_b(po, tag):
                t = ck.tile([128, 128], BF16, tag=tag)
                copy_ps(t, po)
                return t

            P2 = cast_b(mmb(Nb, Mb), "P2")     # M @ M
            P2T = cast_b(mmb(Mb, Nb), "P2T")   # (M @ M)^T
            S4 = ck.tile([128, 128], BF16, tag="S4")
            nc.vector.tensor_add(S4, S2, mmb(P2T, S2))
            P4T = cast_b(mmb(P2, P2T), "P4T")
            if NEUMANN16:
                S8 = ck.tile([128, 128], BF16, tag="S8")
                nc.vector.tensor_add(S8, S4, mmb(P4T, S4))
                P4 = cast_b(mmb(P2T, P2), "P4")
                P8T = cast_b(mmb(P4, P4T), "P8T")
                tinvT = ckx.tile([128, 128], F32R, tag="tinvT")
                nc.vector.tensor_add(tinvT, S8, mmb(P8T, S8))
            else:
                tinvT = ckx.tile([128, 128], F32R, tag="tinvT")
                nc.vector.tensor_add(tinvT, S4, mmb(P4T, S4))

            # --- -W^T = -(Kb^T Tinv^T) ---
            pW = ps.tile([128, 128], F32, tag="ps")
            nc.tensor.matmul(pW, kbr, tinvT.bitcast(F32), start=True, stop=True)
            nWT = ckx.tile([128, 128], F32R, tag="nWT")
            nc.scalar.mul(nWT, pW, -1.0)
            art["h"].append({"knr": knr, "mqk": mqk, "tinvT": tinvT, "nWT": nWT})
        return art

    def stage_b(c, art):
        """S-dependent sequential phase + gated rmsnorm + output projection."""
        vb, sg, qT, csl = art["vb"], art["sg"], art["qT"], art["csl"]
        ofin = cv.tile([128, HPC * HV], F32, tag="ofin")
        for h in range(HPC):
            hsl = slice(h * HV, (h + 1) * HV)
            a = art["h"][h]
            # --- U = Tinv Vb - W S ---
            pU = ps.tile([128, HV], F32, tag="ps")
            _mm(nc, pU, a["nWT"], S[h], start=True, stop=False)
            _mm(nc, pU, a["tinvT"], vb[:, hsl], start=False, stop=True)
            U = cv.tile([128, HV], F32R, tag="U")
            copy_ps(U, pU)

            # --- O = Q S + Mqk U ---
            pO = ps.tile([128, HV], F32, tag="ps")
            _mm(nc, pO, qT[:, h, csl], S[h], start=True, stop=False)
            _mm(nc, pO, a["mqk"], U, start=False, stop=True)

            # --- gated rmsnorm: ofin = (O * rsqrt(mean O^2 + eps)) * silu(g)
            sq2 = scr.tile([128, HV], F32, tag="sq2")
            ms = sm.tile([128, 1], F32, tag="ms")
            nc.scalar.activation(sq2, pO, ACT.Square, accum_out=ms)
            rs1 = sm.tile([128, 1], F32, tag="rs1")
            nc.scalar.activation(rs1, ms, ACT.Sqrt, bias=epst, scale=1.0 / HV)
            rs = sm.tile([128, 1], F32, tag="rs")
            nc.vector.reciprocal(rs, rs1)
            nc.vector.scalar_tensor_tensor(
                out=ofin[:, hsl], in0=pO, scalar=rs, in1=sg[:, hsl],
                op0=AF.mult, op1=AF.mult,
            )

            # --- S += Kn^T U ---
            pD = ps.tile([128, HV], F32, tag="ps")
            _mm(nc, pD, a["knr"], U, start=True, stop=True)
            Sn = sS.tile([128, HV], F32R, tag="S")
            nc.vector.tensor_add(Sn, S[h].bitcast(F32), pD)
            S[h] = Sn

        # --- partial output projection: y[c] = ofin @ woT ---
        oT = otp.tile([128, 4, 128], F32R, tag="oT")
        for s in range(4):
            pt = transpose_f32(ofin[:, s * 128:(s + 1) * 128])
            copy_ps(oT[:, s, :], pt)
        for t2 in range(2):
            py = ps.tile([128, 512], F32, tag="ps")
            for s in range(4):
                _mm(nc, py, oT[:, s, :], woT[:, s, t2 * 512:(t2 + 1) * 512],
                    start=(s == 0), stop=(s == 3))
            yst = cv.tile([128, 512], F32, tag="yst")
            copy_ps(yst, py)
            nc.sync.dma_start(
                out=y[c * 128:(c + 1) * 128, t2 * 512:(t2 + 1) * 512], in_=yst
            )

    # software pipeline: stage A of chunk c+1 is emitted before stage B of
    # chunk c, so the PE always has independent work while the sequential
    # S-chain of the previous chunk waits on DVE/ACT results.
    CPB = LB // C
    arts = {}
    cur = None
    for c in range(NCH + 1):
        if c < NCH:
            if c % CPB == 0:
                cur = stage_lb(c // CPB)
            arts[c] = stage_a(c, *cur)
        if c >= 1:
            stage_b(c - 1, arts.pop(c - 1))

    ctx.close()


_nc_cache = None


def _get_nc():
    global _nc_cache
    if _nc_cache is None:
        _nc_cache = build_program()
    return _nc_cache


def make_in_maps(hidden_states, conv_w, Wq, Wk, Wv, Wb, Wg, Wo, rms_weight):
    arr = lambda a: np.ascontiguousarray(np.asarray(a, dtype=np.float32))
    in_maps = []
    for core in range(N_CORES):
        b, g = core // 2, core % 2
        in_maps.append({
            "hs": arr(hidden_states[b]),
            "cw": arr(conv_w),
            "wq": arr(Wq[g * HPC * HK:(g + 1) * HPC * HK]),
            "wk": arr(Wk[g * HPC * HK:(g + 1) * HPC * HK]),
            "wv": arr(Wv[g * HPC * HV:(g + 1) * HPC * HV]),
            "wb": arr(Wb[g * HPC:(g + 1) * HPC]),
            "wg": arr(Wg[g * HPC * HV:(g + 1) * HPC * HV]),
            "wo": arr(Wo[:, g * HPC * HV:(g + 1) * HPC * HV]),
            "rmsw": arr(rms_weight),
        })
    return in_maps


def unshard(results):
    y = np.empty((B, L, D), np.float32)
    for b in range(B):
        y[b] = results[2 * b]["y"] + results[2 * b + 1]["y"]
    return y


def kernel(hidden_states, conv_w, Wq, Wk, Wv, Wb, Wg, Wo, rms_weight, **_ignored):
    nc = _get_nc()
    in_maps = make_in_maps(hidden_states, conv_w, Wq, Wk, Wv, Wb, Wg, Wo, rms_weight)
    res = run_bass_kernel_spmd(nc, in_maps, core_ids=list(range(N_CORES)))
    return unshard(res.results)



# revision 2
# speedup vs baseline: 2.3886x; 2.3886x over previous
"""DeltaNet forward kernel for Trainium2, sharded over 8 NeuronCores.

Sharding: core c handles batch c//2 and head-pair c%2 (heads {2*(c%2), 2*(c%2)+1}).
Host pre-packs per-core weights: transposed, bf16, q pre-scaled by HK^-0.5 and
rms_weight folded into Wo^T.  The device kernel:
  - DMA-xbar-transposes the bf16 hidden states into SBUF T-layout once,
  - depthwise conv + silu via diagonal-matmul accumulation (diag tiles built once),
  - projections with bf16 operands (FWL weight loads) and N=512 moving tiles,
  - chunked delta-rule (WY representation, chunk=128, Neumann-series (I+A)^-1),
  - gated RMSNorm and a partial output projection against the 512-col Wo slice.
Host sums the two half-DV bf16 partials per batch (row-parallel unshard).
"""

import sys

for _p in ("/opt/trn_rl_repo", "/root/.axon_site"):
    if _p not in sys.path:
        sys.path.insert(0, _p)

import numpy as np
import ml_dtypes

import concourse.bass as bass
import concourse.tile as tile
from concourse import bacc, mybir
from concourse.bass_utils import run_bass_kernel_spmd
from concourse.masks import make_identity

F32 = mybir.dt.float32
BF16 = mybir.dt.bfloat16

B, L, D, H = 4, 2048, 1024, 4
DK, DV = 512, 1024
HK, HV = 128, 256
CONV, EPS = 4, 1e-5
C = 128            # delta-rule chunk length
NCH = L // C       # 16 chunks
XB = 512           # L-block for conv + q projection
CPX = XB // C      # 4 chunks per XB
NXB = L // XB      # 4
KD = D // 128      # 8 contraction slices
HPC = 2            # heads per core
N_CORES = 8
QSCALE = HK ** -0.5
PIPE = 2           # chunk software-pipeline depth


def build_program():
    nc = bacc.Bacc(
        "TRN2", target_bir_lowering=False, debug=False,
        enable_asserts=False, num_devices=N_CORES,
    )

    hsb = nc.dram_tensor("hsb", [L, D], BF16, kind="ExternalInput").ap()
    cwp = nc.dram_tensor("cwp", [128, KD * CONV], F32, kind="ExternalInput").ap()
    wqt = nc.dram_tensor("wqt", [D, HPC * HK], BF16, kind="ExternalInput").ap()
    wkt = nc.dram_tensor("wkt", [D, HPC * HK], BF16, kind="ExternalInput").ap()
    wvbt = nc.dram_tensor("wvbt", [D, 2 * HV + HPC], BF16, kind="ExternalInput").ap()
    wgt = nc.dram_tensor("wgt", [D, HPC * HV], BF16, kind="ExternalInput").ap()
    wot = nc.dram_tensor("wot", [HPC * HV, D], BF16, kind="ExternalInput").ap()
    y = nc.dram_tensor("y", [L, D], BF16, kind="ExternalOutput").ap()

    with tile.TileContext(nc) as tc:
        _build_body(nc, tc, hsb, cwp, wqt, wkt, wvbt, wgt, wot, y)
    nc.compile()
    return nc


def _build_body(nc, tc, hsb, cwp, wqt, wkt, wvbt, wgt, wot, y):
    from contextlib import ExitStack

    AF = mybir.AluOpType
    ACT = mybir.ActivationFunctionType

    ctx = ExitStack()
    const = ctx.enter_context(tc.tile_pool(name="const", bufs=1))
    wp = ctx.enter_context(tc.tile_pool(name="wp", bufs=1))
    hp = ctx.enter_context(tc.tile_pool(name="hp", bufs=1))
    xp = ctx.enter_context(tc.tile_pool(name="xp", bufs=2))
    ck = ctx.enter_context(tc.tile_pool(name="ck", bufs=4))
    nm = ctx.enter_context(tc.tile_pool(name="nm", bufs=3))
    cv = ctx.enter_context(tc.tile_pool(name="cv", bufs=3))
    sS = ctx.enter_context(tc.tile_pool(name="sS", bufs=3))
    ys = ctx.enter_context(tc.tile_pool(name="ys", bufs=3))
    sm = ctx.enter_context(tc.tile_pool(name="sm", bufs=6))
    scr = ctx.enter_context(tc.tile_pool(name="scr", bufs=3))
    ps = ctx.enter_context(tc.tile_pool(name="ps", bufs=3, space="PSUM"))

    # alternate PSUM->SBUF copies between ACT and DVE to balance load
    cp_state = [0]

    def copy_ps(dst, src):
        cp_state[0] ^= 1
        if cp_state[0]:
            nc.scalar.copy(dst, src)
        else:
            nc.vector.tensor_copy(dst, src)

    # ---- constants ----
    identb = const.tile([128, 128], BF16)
    make_identity(nc, identb)
    epst = const.tile([128, 1], F32)
    nc.vector.memset(epst, EPS)
    # umask: 1 where free >= part (upper incl diag); numask: -1 where free > part
    umask = const.tile([128, 128], F32)
    nc.gpsimd.memset(umask, 1.0)
    nc.gpsimd.affine_select(
        out=umask, in_=umask, compare_op=AF.is_ge, fill=0.0,
        base=0, channel_multiplier=-1, pattern=[[1, 128]],
    )
    numask = const.tile([128, 128], F32)
    nc.gpsimd.memset(numask, -1.0)
    nc.gpsimd.affine_select(
        out=numask, in_=numask, compare_op=AF.is_gt, fill=0.0,
        base=0, channel_multiplier=-1, pattern=[[1, 128]],
    )

    # ---- weight loads (already transposed + bf16 on host) ----
    cwt = const.tile([128, KD * CONV], F32)
    nc.sync.dma_start(out=cwt, in_=cwp)
    wqT = wp.tile([128, KD, HPC * HK], BF16)
    nc.sync.dma_start(out=wqT, in_=wqt.rearrange("(k p) c -> p k c", p=128))
    wkT = wp.tile([128, KD, HPC * HK], BF16)
    nc.sync.dma_start(out=wkT, in_=wkt.rearrange("(k p) c -> p k c", p=128))
    wvbT = wp.tile([128, KD, 2 * HV + HPC], BF16)
    nc.sync.dma_start(out=wvbT, in_=wvbt.rearrange("(k p) c -> p k c", p=128))
    wgT = wp.tile([128, KD, HPC * HV], BF16)
    nc.sync.dma_start(out=wgT, in_=wgt.rearrange("(k p) c -> p k c", p=128))
    woT = wp.tile([128, 4, D], BF16)
    nc.sync.dma_start(out=woT, in_=wot.rearrange("(s p) c -> p s c", p=128))

    # conv diag tiles, built once: dg[:, d*4+j, :] = diag(conv_w[d-tile, j])
    dg = const.tile([128, KD * CONV, 128], BF16)
    for dj in range(KD * CONV):
        nc.scalar.mul(dg[:, dj, :], identb, cwt[:, dj:dj + 1])

    # ---- hidden states: DMA xbar transpose into SBUF T-layout ----
    hT = hp.tile([128, KD, L], BF16)
    for d in range(KD):
        nc.sync.dma_start(hT[:, d, :], hsb[:, d * 128:(d + 1) * 128], transpose=True)

    # ---- state ----
    zHV = const.tile([128, HV], BF16)
    nc.vector.memset(zHV, 0.0)
    Sb = []
    for h in range(HPC):
        st = sS.tile([128, HV], BF16, tag="Sb")
        nc.scalar.copy(st, zHV)
        Sb.append(st)

    def stage_xb(xb):
        """conv + silu -> xTb; q projection (T layout) for one 512-token block."""
        x0 = xb * XB
        xTb = xp.tile([128, KD, XB], BF16, tag="xTb")
        for d in range(KD):
            pc = ps.tile([128, XB], F32, tag="acc", bufs=3)
            if xb == 0:
                nc.tensor.matmul(pc, dg[:, d * 4 + 3, :], hT[:, d, 0:XB],
                                 start=True, stop=False)
                for j in range(3):
                    nc.tensor.matmul(
                        pc[:, 3 - j:XB], dg[:, d * 4 + j, :],
                        hT[:, d, 0:XB - (3 - j)],
                        start=False, stop=(j == 2))
            else:
                for j in range(CONV):
                    o = x0 - 3 + j
                    nc.tensor.matmul(pc, dg[:, d * 4 + j, :], hT[:, d, o:o + XB],
                                     start=(j == 0), stop=(j == CONV - 1))
            nc.scalar.activation(xTb[:, d, :], pc, ACT.Silu)

        qTb = xp.tile([128, HPC, XB], BF16, tag="qTb")
        for qt in range(HPC):
            pq = ps.tile([128, XB], F32, tag="acc", bufs=3)
            for ks in range(KD):
                nc.tensor.matmul(pq, wqT[:, ks, qt * 128:(qt + 1) * 128],
                                 xTb[:, ks, :], start=(ks == 0), stop=(ks == KD - 1))
            copy_ps(qTb[:, qt, :], pq)
        return xTb, qTb

    def stage_a(c, xTb, qTb):
        """Chunk-parallel: k/v/g/beta projections (row layout), k-norm, A/Mqk,
        Neumann (I+A)^-T, -W^T."""
        co = (c % CPX) * C
        csl = slice(co, co + C)

        # pair 1: k (row layout) + v-head0 + beta
        pk = ps.tile([128, HPC * HK], F32, tag="acc", bufs=3)
        pv0 = ps.tile([128, HV + HPC], F32, tag="acc", bufs=3)
        for ks in range(KD):
            st = xTb[:, ks, csl]
            nc.tensor.matmul(pk, st, wkT[:, ks, :],
                             start=(ks == 0), stop=(ks == KD - 1))
            nc.tensor.matmul(pv0, st, wvbT[:, ks, 0:HV + HPC],
                             start=(ks == 0), stop=(ks == KD - 1))

        beta = sm.tile([128, HPC], F32, tag="beta")
        nc.scalar.activation(beta, pv0[:, HV:HV + HPC], ACT.Sigmoid)

        # k-norm (free-dim reduce in row layout)
        nsq = sm.tile([128, HPC], F32, tag="nsq")
        sq = scr.tile([128, HPC * HK], F32, tag="sq")
        for h in range(HPC):
            nc.scalar.activation(sq[:, h * HK:(h + 1) * HK],
                                 pk[:, h * HK:(h + 1) * HK],
                                 ACT.Square, accum_out=nsq[:, h:h + 1])
        nrm = sm.tile([128, HPC], F32, tag="nrm")
        nc.scalar.sqrt(nrm, nsq)
        nrm2 = sm.tile([128, HPC], F32, tag="nrm2")
        nc.vector.tensor_scalar_max(nrm2, nrm, 1e-6)
        inv = sm.tile([128, HPC], F32, tag="inv")
        nc.vector.reciprocal(inv, nrm2)

        knr = ck.tile([128, HPC * HK], BF16, tag="knr")
        kbr = ck.tile([128, HPC * HK], BF16, tag="kbr")
        for h in range(HPC):
            hsl = slice(h * HK, (h + 1) * HK)
            nc.vector.tensor_scalar_mul(knr[:, hsl], pk[:, hsl], inv[:, h:h + 1])
            nc.vector.tensor_scalar_mul(kbr[:, hsl], knr[:, hsl], beta[:, h:h + 1])

        vb = cv.tile([128, HPC * HV], BF16, tag="vb")
        nc.vector.tensor_scalar_mul(vb[:, 0:HV], pv0[:, 0:HV], beta[:, 0:1])

        # pair 2: v-head1 + g
        pv1 = ps.tile([128, HV], F32, tag="acc", bufs=3)
        pg = ps.tile([128, HPC * HV], F32, tag="acc", bufs=3)
        for ks in range(KD):
            st = xTb[:, ks, csl]
            nc.tensor.matmul(pv1, st, wvbT[:, ks, HV + HPC:2 * HV + HPC],
                             start=(ks == 0), stop=(ks == KD - 1))
            nc.tensor.matmul(pg, st, wgT[:, ks, :],
                             start=(ks == 0), stop=(ks == KD - 1))
        nc.vector.tensor_scalar_mul(vb[:, HV:2 * HV], pv1, beta[:, 1:2])
        sgb = cv.tile([128, HPC * HV], BF16, tag="sgb")
        nc.scalar.activation(sgb, pg, ACT.Silu)

        art = {"vb": vb, "sgb": sgb, "qTb": qTb, "csl": csl, "knr": knr, "h": []}
        for h in range(HPC):
            hsl = slice(h * HK, (h + 1) * HK)
            pt = ps.tile([128, 128], BF16, tag="fast", bufs=3)
            nc.tensor.transpose(pt, knr[:, hsl], identb)
            knT = ck.tile([128, 128], BF16, tag="knT")
            copy_ps(knT, pt)
            pt2 = ps.tile([128, 128], BF16, tag="fast", bufs=3)
            nc.tensor.transpose(pt2, kbr[:, hsl], identb)
            kbT = ck.tile([128, 128], BF16, tag="kbT")
            copy_ps(kbT, pt2)

            # A^T = Kn Kb^T ; Mqk^T = masked Kn Q^T
            pA = ps.tile([128, 128], F32, tag="fast", bufs=3)
            nc.tensor.matmul(pA, knT, kbT, start=True, stop=True)
            Mb = nm.tile([128, 128], BF16, tag="Mb")
            nc.vector.tensor_mul(Mb, pA, numask)
            pM = ps.tile([128, 128], F32, tag="fast", bufs=3)
            nc.tensor.matmul(pM, knT, qTb[:, h, csl], start=True, stop=True)
            mqk = cv.tile([128, 128], BF16, tag="mqk")
            nc.vector.tensor_mul(mqk, pM, umask)

            # TinvT = sum_{k<16} M^k, M = strict_upper(-A^T), bf16 doubling
            S2 = nm.tile([128, 128], BF16, tag="S2")
            nc.vector.tensor_add(S2, Mb, identb)
            pt3 = ps.tile([128, 128], BF16, tag="fast", bufs=3)
            nc.tensor.transpose(pt3, Mb, identb)
            Nb = nm.tile([128, 128], BF16, tag="Nb")
            copy_ps(Nb, pt3)

            def mmb(lhsT, rhs):
                po = ps.tile([128, 128], F32, tag="fast", bufs=3)
                nc.tensor.matmul(po, lhsT, rhs, start=True, stop=True)
                return po

            def cast_b(po, tag):
                t = nm.tile([128, 128], BF16, tag=tag)
                copy_ps(t, po)
                return t

            P2 = cast_b(mmb(Nb, Mb), "P2")     # M @ M
            P2T = cast_b(mmb(Mb, Nb), "P2T")   # (M @ M)^T
            S4 = nm.tile([128, 128], BF16, tag="S4")
            nc.vector.tensor_add(S4, S2, mmb(P2T, S2))
            P4T = cast_b(mmb(P2, P2T), "P4T")
            S8 = nm.tile([128, 128], BF16, tag="S8")
            nc.vector.tensor_add(S8, S4, mmb(P4T, S4))
            P4 = cast_b(mmb(P2T, P2), "P4")
            P8T = cast_b(mmb(P4, P4T), "P8T")
            tinv = cv.tile([128, 128], BF16, tag="tinv")
            nc.vector.tensor_add(tinv, S8, mmb(P8T, S8))

            # -W^T = -(Kb^T Tinv^T)
            pW = ps.tile([128, 128], F32, tag="fast", bufs=3)
            nc.tensor.matmul(pW, kbr[:, hsl], tinv, start=True, stop=True)
            nWT = cv.tile([128, 128], BF16, tag="nWT")
            nc.scalar.mul(nWT, pW, -1.0)
            art["h"].append({"mqk": mqk, "tinv": tinv, "nWT": nWT})
        return art

    def stage_b(c, art):
        """S-dependent sequential phase + gated rmsnorm + output projection."""
        vb, sgb, qTb, csl, knr = (art["vb"], art["sgb"], art["qTb"], art["csl"],
                                  art["knr"])
        ofin = cv.tile([128, HPC * HV], BF16, tag="ofin")
        for h in range(HPC):
            hsl = slice(h * HV, (h + 1) * HV)
            khsl = slice(h * HK, (h + 1) * HK)
            a = art["h"][h]
            # U = Tinv Vb - W S
            pU = ps.tile([128, HV], F32, tag="sb", bufs=2)
            nc.tensor.matmul(pU, a["nWT"], Sb[h], start=True, stop=False)
            nc.tensor.matmul(pU, a["tinv"], vb[:, hsl], start=False, stop=True)
            Ub = cv.tile([128, HV], BF16, tag="Ub", bufs=2)
            copy_ps(Ub, pU)

            # O = Q S + Mqk U
            pO = ps.tile([128, HV], F32, tag="sb", bufs=2)
            nc.tensor.matmul(pO, qTb[:, h, csl], Sb[h], start=True, stop=False)
            nc.tensor.matmul(pO, a["mqk"], Ub, start=False, stop=True)

            # S += Kn^T U   (bf16 state, fp32 accumulate in PSUM per chunk)
            pD = ps.tile([128, HV], F32, tag="sb", bufs=2)
            nc.tensor.matmul(pD, knr[:, khsl], Ub, start=True, stop=True)
            Sn = sS.tile([128, HV], BF16, tag="Sb")
            nc.vector.tensor_add(Sn, Sb[h], pD)
            Sb[h] = Sn

            # gated rmsnorm: ofin = (O * rsqrt(mean O^2 + eps)) * silu(g)
            sq2 = scr.tile([128, HV], F32, tag="sq")
            ms = sm.tile([128, 1], F32, tag="ms")
            nc.scalar.activation(sq2[:, 0:HV], pO, ACT.Square, accum_out=ms)
            rs1 = sm.tile([128, 1], F32, tag="rs1")
            nc.scalar.activation(rs1, ms, ACT.Sqrt, bias=epst, scale=1.0 / HV)
            rs = sm.tile([128, 1], F32, tag="rs")
            nc.vector.reciprocal(rs, rs1)
            nc.vector.scalar_tensor_tensor(
                out=ofin[:, hsl], in0=pO, scalar=rs, in1=sgb[:, hsl],
                op0=AF.mult, op1=AF.mult,
            )

        # partial output projection: y[c] = ofin @ woT
        oTb = cv.tile([128, 4, 128], BF16, tag="oTb", bufs=2)
        for s in range(4):
            pt = ps.tile([128, 128], BF16, tag="sb", bufs=2)
            nc.tensor.transpose(pt, ofin[:, s * 128:(s + 1) * 128], identb)
            copy_ps(oTb[:, s, :], pt)
        for t2 in range(2):
            py = ps.tile([128, 512], F32, tag="sb", bufs=2)
            for s in range(4):
                nc.tensor.matmul(py, oTb[:, s, :], woT[:, s, t2 * 512:(t2 + 1) * 512],
                                 start=(s == 0), stop=(s == 3))
            yst = ys.tile([128, 512], BF16, tag="yst")
            copy_ps(yst, py)
            nc.sync.dma_start(
                out=y[c * 128:(c + 1) * 128, t2 * 512:(t2 + 1) * 512], in_=yst
            )

    # software pipeline: stage A runs PIPE chunks ahead of the sequential
    # S-chain in stage B so the PE always has independent work queued.
    arts = {}
    cur = None
    for c in range(NCH + PIPE):
        if c < NCH:
            if c % CPX == 0:
                cur = stage_xb(c // CPX)
            arts[c] = stage_a(c, *cur)
        if c >= PIPE:
            stage_b(c - PIPE, arts.pop(c - PIPE))

    ctx.close()


_nc_cache = None


def _get_nc():
    global _nc_cache
    if _nc_cache is None:
        _nc_cache = build_program()
    return _nc_cache


def make_in_maps(hidden_states, conv_w, Wq, Wk, Wv, Wb, Wg, Wo, rms_weight):
    bf = lambda a: np.ascontiguousarray(np.asarray(a, dtype=np.float32)).astype(
        ml_dtypes.bfloat16)
    f32 = lambda a: np.ascontiguousarray(np.asarray(a, dtype=np.float32))

    conv_w = np.asarray(conv_w, np.float32)
    cwp = np.ascontiguousarray(
        conv_w.reshape(KD, 128, CONV).transpose(1, 0, 2).reshape(128, KD * CONV))
    rmsw = np.asarray(rms_weight, np.float32)

    in_maps = []
    for core in range(N_CORES):
        b, g = core // 2, core % 2
        Wq_g = np.asarray(Wq, np.float32)[g * HPC * HK:(g + 1) * HPC * HK]
        Wk_g = np.asarray(Wk, np.float32)[g * HPC * HK:(g + 1) * HPC * HK]
        Wv_g = np.asarray(Wv, np.float32)[g * HPC * HV:(g + 1) * HPC * HV]
        Wb_g = np.asarray(Wb, np.float32)[g * HPC:(g + 1) * HPC]
        Wg_g = np.asarray(Wg, np.float32)[g * HPC * HV:(g + 1) * HPC * HV]
        Wo_g = np.asarray(Wo, np.float32)[:, g * HPC * HV:(g + 1) * HPC * HV]
        # wvbt columns: [v-head0 (256) | beta (2) | v-head1 (256)]
        wvbt = np.concatenate(
            [Wv_g[0:HV].T, Wb_g.T, Wv_g[HV:2 * HV].T], axis=1)
        # fold rms_weight into Wo^T rows
        wot = (Wo_g * np.tile(rmsw, HPC)[None, :]).T
        in_maps.append({
            "hsb": bf(hidden_states[b]),
            "cwp": f32(cwp),
            "wqt": bf(QSCALE * Wq_g.T),
            "wkt": bf(Wk_g.T),
            "wvbt": bf(wvbt),
            "wgt": bf(Wg_g.T),
            "wot": bf(wot),
        })
    return in_maps


def unshard(results):
    out = np.empty((B, L, D), np.float32)
    for b in range(B):
        out[b] = (results[2 * b]["y"].astype(np.float32)
                  + results[2 * b + 1]["y"].astype(np.float32))
    return out


def kernel(hidden_states, conv_w, Wq, Wk, Wv, Wb, Wg, Wo, rms_weight, **_ignored):
    nc = _get_nc()
    in_maps = make_in_maps(hidden_states, conv_w, Wq, Wk, Wv, Wb, Wg, Wo, rms_weight)
    res = run_bass_kernel_spmd(nc, in_maps, core_ids=list(range(N_CORES)))
    return unshard(res.results)
